# revision 11
# baseline (speedup 1.0000x reference)
"""Trainium2 Bass kernel for nn_CircumpunctLevel (8-core SPMD).

Sharding: node axis N=16 -> 2 nodes per core. Each core runs its nodes'
gate/attention/FFN fully on-chip, then an AllGather of the tiny per-node
(C,S,h_tx) phase-resonance vectors couples the 16 nodes; each core adds its
own nodes' field signal to the SBUF-resident residual and stores once.

Math notes (exact simplifications of the reference):
 - Q/K phase rotation is an orthogonal transform applied to both Q and K with
   the same per-head angle => scores are unchanged; skipped.
 - straight-through gate == (logits > 0) in forward; computed as
   0.5*sign(logits)+0.5 on fp32 (exact).
 - phase resonance: cos(ta-tb) = ca*cb + sa*sb with c=(re+eps)/r,
   s=(im+eps)/r, r=sqrt((re+eps)^2+(im+eps)^2)  => r_acc = (C C^T + S S^T)/256.
 - LayerNorm affine (g,b) folded into the following matmul weights host-side.
 - every rsqrt/sigmoid/tanh is computed from Ln/Exp so the scalar engine's
   activation table only toggles between natural_log_exp and gelu (2 loads
   per node-batch unit instead of 5+).
 - softmax denominators come for free from the attention@V matmul: each
   per-head V tile carries an extra all-ones column, so psf row 64 is the
   exp-score sum (kills the separate ones-matmul reduction on PE).

Precision: weights in bf16 (matmul rel err ~4e-3 << 2e-2 gate), activations
bf16 on matmul inputs, fp32/f32r on the residual path. f32r matmuls with
free>=256 run at full PE rate, so bf16 buys DMA/SBUF, not PE cycles.

Perf structure: per-node weights/biases host-packed into two DRAM tensors
(megW bf16, megB f32) laid out exactly as their SBUF tiles; megW is
double-buffered so node 1's weights stream during node 0's compute. x is
loaded and the output stored as one [128, 4*D] DMA per (node, batch); the
residual lives in SBUF (f32r) across the collective. The per-node resonance
vectors are computed before the FFN so the AllGather's ~15us latency hides
under the last node's FFN.
"""

import math

import numpy as np

import concourse.bass as bass
import concourse.bacc as bacc
import concourse.masks as masks
import concourse.mybir as mybir
import concourse.tile as tile
from concourse import bass_utils

F32 = mybir.dt.float32
F32R = mybir.dt.float32r
BF16 = mybir.dt.bfloat16
U16 = mybir.dt.uint16
U32 = mybir.dt.uint32
OP = mybir.AluOpType
AF = mybir.ActivationFunctionType

B, N, T, D, H = 2, 16, 512, 512, 8
DH = D // H  # 64
FF = int(D * (1 + math.sqrt(5)) / 2)  # 828
PR_EPS = 1e-8
LN_EPS = 1e-5
N_CORES = 8
NL = N // N_CORES  # 2 nodes per core
NT = T // 128  # 4 t-blocks
ND = D // 128  # 4 d-blocks
NF = (FF + 127) // 128  # 7 f-blocks (last is 60)
FSZ = [128] * (NF - 1) + [FF - 128 * (NF - 1)]
FOFF = [128 * i for i in range(NF)]
VW = DH + 1  # 65: V columns per head incl the ones column
# per-b layout of the collective payload: C_T [256,2] | S_T [256,2] | htx [2,512]
CC_PER_B = 2 * 256 + 2 * 256 + 2 * 512  # 2048

# megW (bf16) column layout
OFF_WQ = 0
OFF_WK = OFF_WQ + ND * D        # 2048
OFF_WV = OFF_WK + ND * D        # 4096
OFF_WO = OFF_WV + ND * D        # 6144
OFF_WUP = OFF_WO + ND * D       # 8192
OFF_WDN = OFF_WUP + ND * FF     # 11504
OFF_WCN = OFF_WDN + NF * D      # 15088
OFF_CW = OFF_WCN + ND * D       # 17136
WW = OFF_CW + ND * 2            # 17144

# megB (f32) column layout
OFF_QB = 0
OFF_KB = OFF_QB + ND            # 4
OFF_DNB = OFF_KB + ND           # 8
OFF_CNB = OFF_DNB + ND          # 12  (holds 2*center_b for the exp-form tanh)
OFF_UPB = OFF_CNB + ND          # 16
OFF_GB = OFF_UPB + NF           # 23
OFF_CB = OFF_GB + 1             # 24  (holds -commit_b for the exp-form sigmoid)
OFF_GW = OFF_CB + 1             # 25
OFF_VB = OFF_GW + D             # 537
OFF_SEL = OFF_VB + D            # 1049
BW = OFF_SEL + NL * 2           # 1053


def _build(iters: int = 1):
    nc = bacc.Bacc("TRN2", debug=False, num_devices=N_CORES)

    xin = nc.dram_tensor("xin", [B, NL, T, D], F32, kind="ExternalInput")
    megW = nc.dram_tensor("megW", [NL, 128, WW], BF16, kind="ExternalInput")
    megB = nc.dram_tensor("megB", [NL, 128, BW], F32, kind="ExternalInput")
    constM = nc.dram_tensor("constM", [128, 2 * ND * D], F32, kind="ExternalInput")
    out = nc.dram_tensor("out", [B, NL, T, D], F32, kind="ExternalOutput")

    cc_in = nc.dram_tensor("cc_in", [B, CC_PER_B], F32, kind="Internal")
    cc_out = nc.dram_tensor(
        "cc_out", [N_CORES, B, CC_PER_B], F32, kind="Internal", addr_space="Shared"
    )

    with tile.TileContext(nc) as tc:
        with tc.tile_pool(name="const", bufs=1) as cpool, \
             tc.tile_pool(name="wt", bufs=1) as wpool, \
             tc.tile_pool(name="act", bufs=1) as apool, \
             tc.tile_pool(name="ps", bufs=1, space="PSUM") as pp:

            ident = cpool.tile([128, 128], F32, name="ident")
            masks.make_identity(nc, ident[:])
            identR_t = cpool.tile([128, 128], F32R, name="identR")
            nc.scalar.copy(identR_t[:], ident[:])
            identR = identR_t[:]
            ones_r = cpool.tile([128, 2], F32R, name="ones_r")
            nc.gpsimd.memset(ones_r[:].bitcast(U32), 0x3F800000)
            cst = cpool.tile([128, ND * D], F32R, name="cst")
            nc.sync.dma_start(cst[:], constM.ap()[:, 0:ND * D].bitcast(F32R))
            fin_t = [cst[:, k * D:(k + 1) * D] for k in range(ND)]

            # sel tiles in cpool so the gather phase can use them any time
            sel_all = []
            for j in range(NL):
                sj = cpool.tile([N, 2], F32R, name=f"sel{j}")
                sel_all.append(sj[:])

            def body(it):
                sfx = f"_{it}" if iters > 1 else ""
                # ---- DMA issue order: biases first (gate path), then x and
                # weights interleaved so the first QKV matmul starts early ----
                mbs, mws = [], []
                for nl in range(NL):
                    mb = wpool.tile([128, BW], F32, tag="megB", bufs=2,
                                    name=f"megB_{nl}{sfx}")
                    mbs.append(mb)
                    mw = wpool.tile([128, WW], BF16, tag="megW", bufs=2,
                                    name=f"megW_{nl}{sfx}")
                    mws.append(mw)
                nc.sync.dma_start(mbs[0][:], megB.ap()[0])
                for j in range(NL):
                    nc.sync.dma_start(
                        sel_all[j],
                        megB.ap()[0, 0:N, OFF_SEL + 2 * j:OFF_SEL + 2 * (j + 1)].bitcast(F32R))
                xall = {}
                for nl in range(NL):
                    for b in range(B):
                        xa = apool.tile([128, NT * D], F32R, tag=f"xall{nl}{b}",
                                        name=f"xall_{nl}{b}{sfx}")
                        xall[(nl, b)] = xa
                WREG = ((OFF_WQ, OFF_WV), (OFF_WV, OFF_WUP), (OFF_WUP, WW))
                nc.sync.dma_start(
                    xall[(0, 0)][:].rearrange("p (i d) -> p i d", d=D),
                    xin.ap()[0, 0].rearrange("(i p) d -> p i d", p=128).bitcast(F32R))
                nc.sync.dma_start(mws[0][:, OFF_WQ:OFF_WV],
                                  megW.ap()[0, :, OFF_WQ:OFF_WV])
                nc.sync.dma_start(
                    xall[(0, 1)][:].rearrange("p (i d) -> p i d", d=D),
                    xin.ap()[1, 0].rearrange("(i p) d -> p i d", p=128).bitcast(F32R))
                nc.sync.dma_start(mws[0][:, OFF_WV:OFF_WUP],
                                  megW.ap()[0, :, OFF_WV:OFF_WUP])
                for b in range(B):
                    nc.sync.dma_start(
                        xall[(1, b)][:].rearrange("p (i d) -> p i d", d=D),
                        xin.ap()[b, 1].rearrange("(i p) d -> p i d", p=128).bitcast(F32R))
                nc.sync.dma_start(mws[0][:, OFF_WUP:WW],
                                  megW.ap()[0, :, OFF_WUP:WW])
                nc.sync.dma_start(mbs[1][:], megB.ap()[1])
                for lo, hi in WREG:
                    nc.sync.dma_start(mws[1][:, lo:hi], megW.ap()[1, :, lo:hi])

                for nl in range(NL):
                    mw, mb = mws[nl], mbs[nl]
                    wq_t = [mw[:, OFF_WQ + k * D:OFF_WQ + (k + 1) * D] for k in range(ND)]
                    wk_t = [mw[:, OFF_WK + k * D:OFF_WK + (k + 1) * D] for k in range(ND)]
                    wv_t = [mw[:, OFF_WV + k * D:OFF_WV + (k + 1) * D] for k in range(ND)]
                    wo_t = [mw[:, OFF_WO + k * D:OFF_WO + (k + 1) * D] for k in range(ND)]
                    wup_t = [mw[:, OFF_WUP + k * FF:OFF_WUP + (k + 1) * FF] for k in range(ND)]
                    wdn_t = [mw[0:FSZ[k], OFF_WDN + k * D:OFF_WDN + (k + 1) * D] for k in range(NF)]
                    wcn_t = [mw[:, OFF_WCN + k * D:OFF_WCN + (k + 1) * D] for k in range(ND)]
                    cw_c = [mw[:, OFF_CW + 2 * k:OFF_CW + 2 * (k + 1)] for k in range(ND)]
                    qb_c = [mb[:, OFF_QB + k:OFF_QB + k + 1] for k in range(ND)]
                    kb_c = [mb[:, OFF_KB + k:OFF_KB + k + 1] for k in range(ND)]
                    dnb_c = [mb[:, OFF_DNB + k:OFF_DNB + k + 1] for k in range(ND)]
                    cnb_c = [mb[0:128, OFF_CNB + k:OFF_CNB + k + 1] for k in range(ND)]
                    upb_c = [mb[0:FSZ[k], OFF_UPB + k:OFF_UPB + k + 1] for k in range(NF)]
                    gb_c = mb[:, OFF_GB:OFF_GB + 1]
                    cb_t = mb[0:1, OFF_CB:OFF_CB + 1]
                    gw_bc = mb[:, OFF_GW:OFF_GW + D]
                    vb_bc = mb[:, OFF_VB:OFF_VB + D]

                    for b in range(B):
                        u = f"{nl}{b}{sfx}"
                        xa = xall[(nl, b)]
                        xs = [xa[:, i * D:(i + 1) * D] for i in range(NT)]
                        hall = apool.tile([128, NT * D], F32R, tag="hall",
                                          name=f"hall_{u}")
                        hs = [hall[:, i * D:(i + 1) * D] for i in range(NT)]

                        # ---- gate (exact fp32): logits fused mult+reduce ----
                        gate = []
                        gps = pp.tile([2, T], F32, tag="sm", bufs=2, name=f"gps_{u}")
                        for i in range(NT):
                            nc.vector.scalar_tensor_tensor(hs[i], xs[i].bitcast(F32), 1.0,
                                                           gw_bc, OP.mult, OP.mult)
                            lg = apool.tile([128, 1], F32, tag="lgc", bufs=4, name=f"lg{i}_{u}")
                            nc.vector.reduce_sum(lg[:], hs[i].bitcast(F32),
                                                 axis=mybir.AxisListType.X)
                            sg = apool.tile([128, 1], F32, tag="sgc", bufs=4, name=f"sg{i}_{u}")
                            nc.scalar.activation(sg[:], lg[:], AF.Sign, bias=gb_c)
                            g = apool.tile([128, 1], F32, tag=f"gate{i}", name=f"gate{i}_{u}")
                            nc.vector.tensor_scalar(g[:], sg[:], 0.5, 0.5, OP.mult, OP.add)
                            gate.append(g)
                            nc.tensor.transpose(gps[0:1, i * 128:(i + 1) * 128], g[:], ident[:])
                        grow = apool.tile([1, T], F32, tag="grow", name=f"grow_{u}")
                        nc.scalar.copy(grow[:], gps[0:1, :])
                        gate_bc = apool.tile([128, T], F32, tag="rbc", bufs=2, name=f"gatebc_{u}")
                        nc.gpsimd.partition_broadcast(gate_bc[:], grow[:])

                        # ---- LayerNorm: sumsq fused; rsqrt = exp(-0.5*ln(v)) ----
                        def layernorm(tag, uu, norm_engine):
                            s3 = xa[:].rearrange("p (i d) -> p i d", d=D).bitcast(F32)
                            s1 = apool.tile([128, NT], F32, tag="s1", bufs=2, name=f"s1{tag}_{uu}")
                            nc.vector.reduce_sum(s1[:], s3, axis=mybir.AxisListType.X)
                            nc.gpsimd.tensor_tensor(hall[:], xa[:].bitcast(F32),
                                                    xa[:].bitcast(F32), OP.mult)
                            sq = apool.tile([128, NT], F32, tag="sq", bufs=2, name=f"sqs{tag}_{uu}")
                            nc.vector.reduce_sum(
                                sq[:], hall[:].rearrange("p (i d) -> p i d", d=D).bitcast(F32),
                                axis=mybir.AxisListType.X)
                            mu = apool.tile([128, NT], F32, tag="mu", bufs=2, name=f"mu{tag}_{uu}")
                            nc.vector.tensor_scalar(mu[:], s1[:], 1.0 / D, None, OP.mult)
                            m2 = apool.tile([128, NT], F32, tag="m2e", bufs=2, name=f"m2e{tag}_{uu}")
                            nc.vector.tensor_tensor(m2[:], mu[:], mu[:], OP.mult)
                            ve = apool.tile([128, NT], F32, tag="ve", bufs=2, name=f"ve{tag}_{uu}")
                            nc.vector.tensor_scalar(ve[:], sq[:], 1.0 / D, LN_EPS, OP.mult, OP.add)
                            nc.vector.tensor_tensor(ve[:], ve[:], m2[:], OP.subtract)
                            lv = apool.tile([128, NT], F32, tag="lv", bufs=2, name=f"lv{tag}_{uu}")
                            nc.scalar.activation(lv[:], ve[:], AF.Ln)
                            rs = apool.tile([128, NT], F32, tag="rs", bufs=2, name=f"rs{tag}_{uu}")
                            nc.scalar.activation(rs[:], lv[:], AF.Exp, scale=-0.5)
                            eng = nc.gpsimd if norm_engine == "pool" else nc.vector
                            for i in range(NT):
                                eng.tensor_scalar(
                                    hs[i], xs[i], mu[:, i:i + 1], rs[:, i:i + 1],
                                    OP.subtract, OP.mult)

                        def transpose_fm(tag, uu, copy_engines):
                            # token-major [t,d] residual -> feature-major bf16 [d,t]
                            res = []
                            for dblk in range(ND):
                                ps = pp.tile([128, T], F32, tag="tr", bufs=2, name=f"tp{tag}{dblk}_{uu}")
                                for i in range(NT):
                                    nc.tensor.matmul(
                                        ps[:, i * 128:(i + 1) * 128].bitcast(F32R),
                                        hall[:, i * D + dblk * 128:i * D + (dblk + 1) * 128],
                                        identR, is_transpose=True)
                                tt = apool.tile([128, T], BF16, tag=f"hT{dblk}", name=f"hT{tag}{dblk}_{uu}")
                                if copy_engines[dblk % len(copy_engines)] == "act":
                                    nc.scalar.copy(tt[:], ps[:])
                                else:
                                    nc.vector.tensor_scalar(tt[:], ps[:], 1.0, None, OP.mult)
                                res.append(tt)
                            return res

                        layernorm("a", u, "pool")
                        hT = transpose_fm("a", u, ("act",))

                        # ---- Q,K feature-major; V token-major (with ones col) ----
                        QT, KT = [], []
                        for e in range(ND):
                            sl = slice(e * 128, (e + 1) * 128)
                            psq = pp.tile([128, T], F32, tag="mm", bufs=2, name=f"psq{e}_{u}")
                            for k in range(ND):
                                nc.tensor.matmul(psq[:], wq_t[k][:, sl], hT[k][:],
                                                 start=(k == 0), stop=(k == ND - 1))
                            qt = apool.tile([128, T], BF16, tag=f"QT{e}", name=f"QT{e}_{u}")
                            nc.scalar.activation(qt[:], psq[:], AF.Identity, bias=qb_c[e])
                            QT.append(qt)
                            psk = pp.tile([128, T], F32, tag="mm", bufs=2, name=f"psk{e}_{u}")
                            for k in range(ND):
                                nc.tensor.matmul(psk[:], wk_t[k][:, sl], hT[k][:],
                                                 start=(k == 0), stop=(k == ND - 1))
                            kt = apool.tile([128, T], BF16, tag=f"KT{e}", name=f"KT{e}_{u}")
                            nc.vector.scalar_tensor_tensor(kt[:], psk[:], kb_c[e], gate_bc[:],
                                                           OP.add, OP.mult)
                            KT.append(kt)
                        Vn = []
                        for i in range(NT):
                            sl = slice(i * 128, (i + 1) * 128)
                            psv = pp.tile([128, D], F32, tag="mm", bufs=2, name=f"psv{i}_{u}")
                            for k in range(ND):
                                nc.tensor.matmul(psv[:], hT[k][:, sl], wv_t[k][:],
                                                 start=(k == 0), stop=(k == ND - 1))
                            vn = apool.tile([128, H * VW], BF16, tag=f"Vn{i}", name=f"Vn{i}_{u}")
                            vh = vn[:].rearrange("p (h c) -> p h c", c=VW)
                            nc.gpsimd.memset(vh[:, :, DH:VW].bitcast(U16), 0x3F80)
                            nc.vector.tensor_tensor(
                                vh[:, :, 0:DH], psv[:].rearrange("p (h c) -> p h c", c=DH),
                                vb_bc.rearrange("p (h c) -> p h c", c=DH), OP.add)
                            Vn.append(vn)

                        # ---- attention; denominator rides in psf row 64 ----
                        fT = [apool.tile([128, T], BF16, tag=f"fT{k}", name=f"fT{k}_{u}") for k in range(ND)]
                        for hh in range(H):
                            qrow = hh // 2
                            roff = (hh % 2) * DH
                            rsl = slice(roff, roff + DH)
                            pTs = []
                            for s in range(NT):
                                pss = pp.tile([128, T], F32, tag="sc", bufs=2, name=f"sc{hh}{s}_{u}")
                                nc.tensor.matmul(pss[:], KT[qrow][rsl, s * 128:(s + 1) * 128],
                                                 QT[qrow][rsl, :], start=True, stop=True)
                                pt = apool.tile([128, T], BF16, tag=f"pT{s}", bufs=2, name=f"pT{hh}{s}_{u}")
                                nc.scalar.activation(pt[:], pss[:], AF.Exp, scale=1.0 / math.sqrt(DH))
                                pTs.append(pt)
                            psf = pp.tile([VW, T], F32, tag="mm", bufs=2, name=f"psf{hh}_{u}")
                            for s in range(NT):
                                nc.tensor.matmul(psf[:], Vn[s][:, hh * VW:(hh + 1) * VW], pTs[s][:],
                                                 start=(s == 0), stop=(s == NT - 1))
                            rr = apool.tile([1, T], F32, tag="rr", bufs=2, name=f"rr{hh}_{u}")
                            nc.vector.reciprocal(rr[:], psf[DH:VW, :])
                            rbc = apool.tile([DH, T], F32, tag="rbc", bufs=2, name=f"rbc{hh}_{u}")
                            nc.gpsimd.partition_broadcast(rbc[:], rr[:])
                            nc.vector.tensor_tensor(fT[qrow][rsl, :], psf[0:DH, :], rbc[:], OP.mult)

                        # ---- wo, transpose, x1 = x + field*gate (in-place) ----
                        oS = []
                        for e in range(ND):
                            sl = slice(e * 128, (e + 1) * 128)
                            pso = pp.tile([128, T], F32, tag="mm", bufs=2, name=f"pso{e}_{u}")
                            for k in range(ND):
                                nc.tensor.matmul(pso[:], wo_t[k][:, sl], fT[k][:],
                                                 start=(k == 0), stop=(k == ND - 1))
                            os_ = apool.tile([128, T], F32R, tag=f"oS{e}", name=f"oS{e}_{u}")
                            nc.vector.tensor_scalar(os_[:], pso[:], 1.0, None, OP.mult)
                            oS.append(os_)
                        for i in range(NT):
                            pst = pp.tile([128, D], F32, tag="tr", bufs=2, name=f"fot{i}_{u}")
                            for e in range(ND):
                                nc.tensor.matmul(pst[:, e * 128:(e + 1) * 128].bitcast(F32R),
                                                 oS[e][:, i * 128:(i + 1) * 128], identR,
                                                 is_transpose=True)
                            nc.vector.scalar_tensor_tensor(xs[i], pst[:], gate[i][:], xs[i],
                                                           OP.mult, OP.add)

                        # ---- center pool (sum over t) straight off the f32r residual ----
                        cpc = []
                        for dblk in range(ND):
                            psc = pp.tile([128, 2], F32, tag="sm", bufs=2, name=f"cp{dblk}_{u}")
                            for i in range(NT):
                                nc.tensor.matmul(psc[:], xs[i][:, dblk * 128:(dblk + 1) * 128],
                                                 ones_r[:], start=(i == 0), stop=(i == NT - 1))
                            cc = apool.tile([128, 2], BF16, tag=f"cpc{dblk}", name=f"cpc{dblk}_{u}")
                            nc.scalar.copy(cc[:], psc[:])
                            cpc.append(cc)

                        # ---- commit: sigmoid(z) = 1/(1+exp(-z)) on the exp table ----
                        psd = pp.tile([2, 2], F32, tag="sm", bufs=2, name=f"cd_{u}")
                        for dblk in range(ND):
                            nc.tensor.matmul(psd[:], cpc[dblk][:], cw_c[dblk],
                                             start=(dblk == 0), stop=(dblk == ND - 1))
                        cme = apool.tile([1, 1], F32, tag="cme", name=f"cme_{u}")
                        nc.scalar.activation(cme[:], psd[0:1, 0:1], AF.Exp, scale=-1.0, bias=cb_t)
                        cmt = apool.tile([1, 1], F32, tag="cmt", name=f"cmt_{u}")
                        nc.vector.tensor_scalar(cmt[:], cme[:], 1.0, None, OP.add)
                        nc.vector.reciprocal(cmt[:], cmt[:])
                        cmt_bc = apool.tile([128, 1], F32, tag="cmtbc", name=f"cmtbc_{u}")
                        nc.gpsimd.partition_broadcast(cmt_bc[:], cmt[:])

                        # ---- center: tanh(z) = 1 - 2/(exp(2z)+1) on the exp table ----
                        cen = []
                        for e in range(ND):
                            sl = slice(e * 128, (e + 1) * 128)
                            pse = pp.tile([128, 2], F32, tag="sm", bufs=2, name=f"ce{e}_{u}")
                            for k in range(ND):
                                nc.tensor.matmul(pse[:], wcn_t[k][:, sl], cpc[k][:],
                                                 start=(k == 0), stop=(k == ND - 1))
                            ce = apool.tile([128, 2], F32R, tag=f"cen{e}", name=f"cen{e}_{u}")
                            cet = apool.tile([128, 2], F32, tag="cet", bufs=2, name=f"cet{e}_{u}")
                            nc.scalar.activation(cet[:], pse[:], AF.Exp, scale=2.0, bias=cnb_c[e])
                            nc.vector.tensor_scalar(cet[:], cet[:], 1.0, None, OP.add)
                            nc.vector.reciprocal(cet[:], cet[:])
                            nc.vector.tensor_scalar(ce[:], cet[:], -2.0, 1.0, OP.mult, OP.add)
                            cen.append(ce)
                        txc = []
                        for e in range(ND):
                            tx = apool.tile([128, 2], F32R, tag=f"txc{e}", name=f"txc{e}_{u}")
                            nc.vector.tensor_scalar(tx[:], cen[e][:], cmt_bc[:], None, OP.mult)
                            txc.append(tx)
                        psh = pp.tile([2, D], F32, tag="sm", bufs=2, name=f"hres_{u}")
                        for k in range(ND):
                            nc.tensor.matmul(psh[:], cen[k][:], fin_t[k],
                                             start=(k == 0), stop=(k == ND - 1))
                        hres = apool.tile([1, D], F32, tag="hres", name=f"hres_{u}")
                        nc.vector.tensor_scalar(hres[:], psh[0:1, :], 1.0, None, OP.mult)
                        psx = pp.tile([2, D], F32, tag="sm", bufs=2, name=f"htx_{u}")
                        for k in range(ND):
                            nc.tensor.matmul(psx[:], txc[k][:], fin_t[k],
                                             start=(k == 0), stop=(k == ND - 1))
                        htx = apool.tile([1, D], F32, tag="htx", name=f"htx_{u}")
                        nc.scalar.copy(htx[:], psx[0:1, :])
                        nc.sync.dma_start(cc_in.ap()[b, 1024 + nl * D:1024 + (nl + 1) * D].unsqueeze(0), htx[:])

                        # ---- resonance C/S rows; 1/r = exp(-0.5*ln(r2)) ----
                        hv = hres[:].rearrange("p (c two) -> p two c", two=2)
                        ree = apool.tile([1, 256], F32, tag="ree", name=f"ree_{u}")
                        nc.gpsimd.tensor_scalar(ree[:], hv[:, 0, :], PR_EPS, None, OP.add)
                        ime = apool.tile([1, 256], F32, tag="ime", name=f"ime_{u}")
                        nc.gpsimd.tensor_scalar(ime[:], hv[:, 1, :], PR_EPS, None, OP.add)
                        r2 = apool.tile([1, 256], F32, tag="r2", name=f"r2_{u}")
                        nc.gpsimd.tensor_tensor(r2[:], ree[:], ree[:], OP.mult)
                        r2b = apool.tile([1, 256], F32, tag="r2b", name=f"r2b_{u}")
                        nc.gpsimd.tensor_tensor(r2b[:], ime[:], ime[:], OP.mult)
                        nc.gpsimd.tensor_tensor(r2[:], r2[:], r2b[:], OP.add)
                        lr2 = apool.tile([1, 256], F32, tag="r2b", name=f"lr2_{u}")
                        nc.scalar.activation(lr2[:], r2[:], AF.Ln)
                        ri = apool.tile([1, 256], F32, tag="rr", bufs=2, name=f"ri_{u}")
                        nc.scalar.activation(ri[:], lr2[:], AF.Exp, scale=-0.5)
                        nc.gpsimd.tensor_tensor(ree[:], ree[:], ri[:], OP.mult)
                        nc.gpsimd.tensor_tensor(ime[:], ime[:], ri[:], OP.mult)
                        cv = cc_in.ap()[b, 0:512].unsqueeze(0).rearrange("b (p two) -> b two p", two=2)
                        nc.sync.dma_start(cv[:, nl, :], ree[:])
                        sv = cc_in.ap()[b, 512:1024].unsqueeze(0).rearrange("b (p two) -> b two p", two=2)
                        nc.sync.dma_start(sv[:, nl, :], ime[:])

                        # ---- LN2, FFN (gelu table), x2 = x1 + commit*gate*ff ----
                        layernorm("b", u, "pool")
                        h2T = transpose_fm("b", u, ("dve",))
                        upT = []
                        for f in range(NF):
                            fsl = slice(FOFF[f], FOFF[f] + FSZ[f])
                            psu = pp.tile([FSZ[f], T], F32, tag="sc", bufs=2, name=f"psu{f}_{u}")
                            for k in range(ND):
                                nc.tensor.matmul(psu[:], wup_t[k][:, fsl], h2T[k][:],
                                                 start=(k == 0), stop=(k == ND - 1))
                            ut = apool.tile([FSZ[f], T], BF16, tag=f"pT{f % 4}", bufs=2, name=f"upT{f}_{u}")
                            nc.scalar.activation(ut[:], psu[:], AF.Gelu, bias=upb_c[f])
                            upT.append(ut)
                        ffT = []
                        for e in range(ND):
                            sl = slice(e * 128, (e + 1) * 128)
                            psn = pp.tile([128, T], F32, tag="mm", bufs=2, name=f"psn{e}_{u}")
                            for k in range(NF):
                                nc.tensor.matmul(psn[:], wdn_t[k][:, sl], upT[k][:],
                                                 start=(k == 0), stop=(k == NF - 1))
                            ft = apool.tile([128, T], F32R, tag=f"oS{e}", name=f"ffT{e}_{u}")
                            nc.vector.tensor_scalar(ft[:], psn[:], dnb_c[e], None, OP.add)
                            ffT.append(ft)
                        for i in range(NT):
                            pst2 = pp.tile([128, D], F32, tag="tr", bufs=2, name=f"fft{i}_{u}")
                            for e in range(ND):
                                nc.tensor.matmul(pst2[:, e * 128:(e + 1) * 128].bitcast(F32R),
                                                 ffT[e][:, i * 128:(i + 1) * 128], identR,
                                                 is_transpose=True)
                            gc = apool.tile([128, 1], F32, tag="gc", bufs=4, name=f"gc{i}_{u}")
                            nc.vector.tensor_tensor(gc[:], gate[i][:], cmt_bc[:], OP.mult)
                            nc.vector.scalar_tensor_tensor(xs[i], pst2[:], gc[:], xs[i],
                                                           OP.mult, OP.add)

                # ---- AllGather (latency hides under the last node's FFN) ----
                nc.gpsimd.collective_compute(
                    "AllGather", OP.bypass, replica_groups=[list(range(N_CORES))],
                    ins=[cc_in.ap()], outs=[cc_out.ap()],
                )
                # fow loaded into the hall scratch (dead after the last LN2)
                fow_s = apool.tile([128, NT * D], F32R, tag="hall", name=f"fow_g{sfx}")
                nc.sync.dma_start(fow_s[:], constM.ap()[:, ND * D:2 * ND * D].bitcast(F32R))
                fow_t = [fow_s[:, k * D:(k + 1) * D] for k in range(ND)]
                for b in range(B):
                    u = f"g{b}{sfx}"
                    CT, ST = [], []
                    for half in range(2):
                        psl = slice(half * 128, (half + 1) * 128)
                        ct = apool.tile([128, N], F32R, tag=f"CT{half}", name=f"CT{half}_{u}")
                        cgv = cc_out.ap()[:, b:b + 1, 0:512].rearrange(
                            "r b (p two) -> b p r two", two=2)
                        nc.sync.dma_start(ct[:].rearrange("p (r two) -> p r two", two=2),
                                          cgv[0, psl, :, :].bitcast(F32R))
                        CT.append(ct)
                        st = apool.tile([128, N], F32R, tag=f"ST{half}", name=f"ST{half}_{u}")
                        sgv = cc_out.ap()[:, b:b + 1, 512:1024].rearrange(
                            "r b (p two) -> b p r two", two=2)
                        nc.sync.dma_start(st[:].rearrange("p (r two) -> p r two", two=2),
                                          sgv[0, psl, :, :].bitcast(F32R))
                        ST.append(st)
                    htxg = apool.tile([N, D], F32R, tag="htxg", name=f"htxg_{u}")
                    hgv = cc_out.ap()[:, b:b + 1, 1024:2048].rearrange(
                        "r b (nl e) -> b r nl e", nl=2)
                    for r in range(N_CORES):
                        nc.sync.dma_start(htxg[r * 2:(r + 1) * 2, :],
                                          hgv[0, r, :, :].bitcast(F32R))

                    psr_ = pp.tile([N, N], F32, tag="sm", bufs=2, name=f"racc_{u}")
                    nc.tensor.matmul(psr_[:], CT[0][:], CT[0][:], start=True, stop=False)
                    nc.tensor.matmul(psr_[:], CT[1][:], CT[1][:], start=False, stop=False)
                    nc.tensor.matmul(psr_[:], ST[0][:], ST[0][:], start=False, stop=False)
                    nc.tensor.matmul(psr_[:], ST[1][:], ST[1][:], start=False, stop=True)
                    rsb = apool.tile([N, N], F32R, tag="rsb", name=f"rsb_{u}")
                    nc.scalar.activation(rsb[:], psr_[:], AF.Copy, scale=1.0 / 256.0)

                    for nl in range(NL):
                        uu = f"{u}n{nl}"
                        psl_ = pp.tile([N, 2], F32, tag="sm", bufs=2, name=f"rloc_{uu}")
                        nc.tensor.matmul(psl_[:], rsb[:], sel_all[nl], start=True, stop=True)
                        rloc = apool.tile([N, 2], F32R, tag="rloc", name=f"rloc_{uu}")
                        nc.scalar.copy(rloc[:], psl_[:])
                        rcv = []
                        for e in range(ND):
                            psv_ = pp.tile([128, 2], F32, tag="sm", bufs=2, name=f"rcv{e}_{uu}")
                            nc.tensor.matmul(psv_[:], htxg[:, e * 128:(e + 1) * 128],
                                             rloc[:], start=True, stop=True)
                            rv = apool.tile([128, 2], F32R, tag=f"rcv{e}", name=f"rcv{e}_{uu}")
                            nc.scalar.copy(rv[:], psv_[:])
                            rcv.append(rv)
                        psfs = pp.tile([2, D], F32, tag="sm", bufs=2, name=f"fs_{uu}")
                        for e in range(ND):
                            nc.tensor.matmul(psfs[:], rcv[e][:], fow_t[e],
                                             start=(e == 0), stop=(e == ND - 1))
                        fsr = apool.tile([1, D], F32, tag="fsr", name=f"fsr_{uu}")
                        nc.scalar.copy(fsr[:], psfs[0:1, :])
                        fbc = apool.tile([128, D], F32, tag="rbc", bufs=2, name=f"fbc_{uu}")
                        nc.gpsimd.partition_broadcast(fbc[:], fsr[:])
                        xa = xall[(nl, b)]
                        for i in range(NT):
                            eng = nc.gpsimd if i % 2 == 0 else nc.vector
                            eng.tensor_tensor(xa[:, i * D:(i + 1) * D],
                                              xa[:, i * D:(i + 1) * D], fbc[:], OP.add)
                        nc.sync.dma_start(
                            out.ap()[b, nl].rearrange("(i p) d -> p i d", p=128),
                            xa[:].rearrange("p (i d) -> p i d", d=D).bitcast(F32))

            for it in range(iters):
                body(it)

    nc.compile()
    return nc


def _prep_inputs(inputs):
    """Host-side preprocessing + sharding. Returns in_maps for the 8 cores."""
    import ml_dtypes
    f = lambda k: np.asarray(inputs[k], np.float32)
    x = f("x")
    ln1_g, ln1_b = f("ln1_g"), f("ln1_b")
    ln2_g, ln2_b = f("ln2_g"), f("ln2_b")
    wq = ln1_g[:, :, None] * f("wq")
    wk = ln1_g[:, :, None] * f("wk")
    wv = ln1_g[:, :, None] * f("wv")
    qb = np.einsum("nd,nde->ne", ln1_b, f("wq"))
    kbv = np.einsum("nd,nde->ne", ln1_b, f("wk"))
    vbv = np.einsum("nd,nde->ne", ln1_b, f("wv"))
    wo = f("wo")
    wup = ln2_g[:, :, None] * f("up_w")
    upb = f("up_b") + np.einsum("nd,ndf->nf", ln2_b, f("up_w"))
    wdn = f("down_w")
    dnb = f("down_b")
    wcn = f("center_w") / T
    cnb = 2.0 * f("center_b")          # exp-form tanh wants 2*(z+b)
    cwv = np.repeat((f("commit_w") / T)[:, :, None], 2, axis=2)
    fin = f("field_in_w")
    fow = f("field_out_w") * float(np.asarray(inputs["conductance"], np.float32))
    gate_w = f("gate_w")
    gate_b = f("gate_b")
    commit_b = -f("commit_b")          # exp-form sigmoid wants exp(-(z+b))

    def tile128(M):
        # [nb*128, C] -> [128, nb*C] with column block k = rows 128k..128k+128
        nb, C = M.shape[0] // 128, M.shape[1]
        return np.ascontiguousarray(
            M.reshape(nb, 128, C).transpose(1, 0, 2).reshape(128, nb * C))

    def col4(v):
        # [512] -> [128, 4]
        return np.ascontiguousarray(v.reshape(ND, 128).T)

    constM = np.concatenate([tile128(fin), tile128(fow)], axis=1)

    in_maps = []
    for c in range(N_CORES):
        ns = slice(c * NL, (c + 1) * NL)
        mw = np.zeros((NL, 128, WW), np.float32)
        mb = np.zeros((NL, 128, BW), np.float32)
        for j in range(NL):
            n = c * NL + j
            mw[j, :, OFF_WQ:OFF_WK] = tile128(wq[n])
            mw[j, :, OFF_WK:OFF_WV] = tile128(wk[n])
            mw[j, :, OFF_WV:OFF_WO] = tile128(wv[n])
            mw[j, :, OFF_WO:OFF_WUP] = tile128(wo[n])
            mw[j, :, OFF_WUP:OFF_WDN] = tile128(wup[n])
            dpad = np.zeros((NF * 128, D), np.float32)
            dpad[:FF] = wdn[n]
            mw[j, :, OFF_WDN:OFF_WCN] = tile128(dpad)
            mw[j, :, OFF_WCN:OFF_CW] = tile128(wcn[n])
            mw[j, :, OFF_CW:WW] = tile128(cwv[n])
            mb[j, :, OFF_QB:OFF_KB] = col4(qb[n])
            mb[j, :, OFF_KB:OFF_DNB] = col4(kbv[n])
            mb[j, :, OFF_DNB:OFF_CNB] = col4(dnb[n])
            mb[j, :, OFF_CNB:OFF_UPB] = col4(cnb[n])
            upad = np.zeros(NF * 128, np.float32)
            upad[:FF] = upb[n]
            mb[j, :, OFF_UPB:OFF_GB] = upad.reshape(NF, 128).T
            mb[j, :, OFF_GB] = gate_b[n]
            mb[j, :, OFF_CB] = commit_b[n]
            mb[j, :, OFF_GW:OFF_VB] = gate_w[n][None, :]
            mb[j, :, OFF_VB:OFF_SEL] = vbv[n][None, :]
            for jj in range(NL):
                mb[j, c * NL + jj, OFF_SEL + 2 * jj:OFF_SEL + 2 * (jj + 1)] = 1.0
        in_maps.append({
            "xin": np.ascontiguousarray(x[:, ns]),
            "megW": mw.astype(ml_dtypes.bfloat16),
            "megB": mb,
            "constM": constM,
        })
    return in_maps


_NC_CACHE = {}


def _get_nc(iters=1):
    if iters not in _NC_CACHE:
        _NC_CACHE[iters] = _build(iters)
    return _NC_CACHE[iters]


def kernel(**inputs):
    nc = _get_nc()
    in_maps = _prep_inputs(inputs)
    res = bass_utils.run_bass_kernel_spmd(nc, in_maps, core_ids=list(range(N_CORES)))
    full = np.empty((B, N, T, D), np.float32)
    for c in range(N_CORES):
        full[:, c * NL:(c + 1) * NL] = res.results[c]["out"]
    return full
